# revision 1
# baseline (speedup 1.0000x reference)
"""Trainium2 Bass kernel for MemoryEfficientMultiHeadAttention (8 NeuronCores).

Sharding: hybrid data/tensor parallel. Core c handles batch b = c//2 and head
group half = c%2 (8 of 16 heads, i.e. 512 of 1024 qkv features). Each core:
  q,k  = (x_b @ w.T + b) in [feat, tok] layout (feat on partitions)
  vT   = (x_b @ wv.T + b) in [tok, feat] layout
  per head: scoresT = k_h.T @ q_h (transposed scores, [kt, qt])
            PT = exp(scoresT / 8)            (no max-subtraction: scores are O(1))
            attU.T += vT_h.T @ PT            (accumulate over kt tiles)
            denom  += ones.T @ PT            (row sums via M=1 matmuls)
  attS = attU * (1/denom)  broadcast via K=2 selector matmul
  outp = attS.T @ dense_w_slice.T            (partial over this core's 512 feats)
Host: out[b] = outp[2b] + outp[2b+1] + dense_b.

All matmuls run in bf16 (1 cycle/row on TRN2 PE; fp32 is 4 cycles/row) with
fp32 PSUM accumulation.
"""

import sys
import time
from contextlib import ExitStack

import numpy as np

try:
    import concourse.bass as bass  # noqa: F401
except ImportError:  # pragma: no cover
    sys.path.insert(0, "/opt/trn_rl_repo")

import ml_dtypes

import concourse.bacc as bacc
import concourse.mybir as mybir
import concourse.tile as tile

P = 128
BF16 = mybir.dt.bfloat16
F32 = mybir.dt.float32
NPBF16 = ml_dtypes.bfloat16

B, S, D = 4, 2048, 1024
HHALF = 512  # features per core (8 heads x 64)

# head-selector for the denominator broadcast matmul: row0 -> head A cols,
# row1 -> head B cols
_SEL2 = np.zeros((2, P), NPBF16)
_SEL2[0, 0:64] = 1
_SEL2[1, 64:128] = 1


def _build_nc(loop_r=None):
    nc = bacc.Bacc()

    xT = nc.dram_tensor("xT", [D, S], BF16, kind="ExternalInput")
    wqT = nc.dram_tensor("wqT", [D, HHALF], BF16, kind="ExternalInput")
    wkT = nc.dram_tensor("wkT", [D, HHALF], BF16, kind="ExternalInput")
    wvT = nc.dram_tensor("wvT", [D, HHALF], BF16, kind="ExternalInput")
    dwT = nc.dram_tensor("dwT", [HHALF, D], BF16, kind="ExternalInput")
    qb = nc.dram_tensor("qb", [P, 4], F32, kind="ExternalInput")
    kb = nc.dram_tensor("kb", [P, 4], F32, kind="ExternalInput")
    vb = nc.dram_tensor("vb", [P, HHALF], BF16, kind="ExternalInput")
    sel = nc.dram_tensor("sel", [2, P], BF16, kind="ExternalInput")
    outp = nc.dram_tensor("outp", [S, D], F32, kind="ExternalOutput")

    Exp = mybir.ActivationFunctionType.Exp

    with tile.TileContext(nc) as tc, ExitStack() as ctx:
        wpool = ctx.enter_context(tc.tile_pool(name="weights", bufs=1))
        spool = ctx.enter_context(tc.tile_pool(name="state", bufs=1))
        ptpool = ctx.enter_context(tc.tile_pool(name="pt", bufs=3))
        evpool = ctx.enter_context(tc.tile_pool(name="evac", bufs=4))
        ps_sc = ctx.enter_context(tc.tile_pool(name="pssc", bufs=2, space="PSUM"))
        ps_acc = ctx.enter_context(tc.tile_pool(name="psacc", bufs=2, space="PSUM"))
        ps_misc = ctx.enter_context(tc.tile_pool(name="psmisc", bufs=2, space="PSUM"))

        # ---- persistent SBUF state (loaded once) ----
        xT_sb = wpool.tile([P, 8, S], BF16)
        nc.sync.dma_start(xT_sb[:], xT.rearrange("(o p) t -> p o t", p=P))
        wqT_sb = wpool.tile([P, 8, HHALF], BF16)
        nc.sync.dma_start(wqT_sb[:], wqT.rearrange("(o p) f -> p o f", p=P))
        wkT_sb = wpool.tile([P, 8, HHALF], BF16)
        nc.sync.dma_start(wkT_sb[:], wkT.rearrange("(o p) f -> p o f", p=P))
        wvT_sb = wpool.tile([P, 8, HHALF], BF16)
        nc.sync.dma_start(wvT_sb[:], wvT.rearrange("(o p) f -> p o f", p=P))
        dwT_sb = wpool.tile([P, 4, D], BF16)
        nc.sync.dma_start(dwT_sb[:], dwT.rearrange("(o p) f -> p o f", p=P))
        qb_sb = wpool.tile([P, 4], F32)
        nc.sync.dma_start(qb_sb[:], qb[:])
        kb_sb = wpool.tile([P, 4], F32)
        nc.sync.dma_start(kb_sb[:], kb[:])
        vb_sb = wpool.tile([P, HHALF], BF16)
        nc.sync.dma_start(vb_sb[:], vb[:])
        sel2 = wpool.tile([2, P], BF16)
        nc.sync.dma_start(sel2[:], sel[:])
        onesk = wpool.tile([P, 1], BF16)
        nc.vector.memset(onesk[:], 1.0)

        q_sb = spool.tile([P, 4, S], BF16)
        k_sb = spool.tile([P, 4, S], BF16)
        vT_sb = spool.tile([P, 16, HHALF], BF16)
        attU_sb = spool.tile([P, 4, S], BF16)
        stage_sb = spool.tile([P, 16, 512], BF16)  # denoms at rows {0,32}
        d32_sb = spool.tile([32, 512], BF16)
        r32_sb = spool.tile([32, 512], BF16)
        r2_sb = spool.tile([2, 16, 512], BF16)

        def v_proj(t):
            ps = ps_acc.tile([P, 512], F32, tag="acc")
            for kk in range(8):
                nc.tensor.matmul(
                    ps[:],
                    lhsT=xT_sb[:, kk, t * 128 : (t + 1) * 128],
                    rhs=wvT_sb[:, kk, :],
                    start=(kk == 0),
                    stop=(kk == 7),
                )
            nc.vector.tensor_add(vT_sb[:, t, :], ps[:], vb_sb[:])

        def body():
            # ---- per head-pair: q/k projection then attention ----
            # (V projection is interleaved into the first pair's first kt loop
            # so the ACT engine starts exp work as early as possible.)
            for p in range(4):
                for t4 in range(4):
                    tok = slice(t4 * 512, (t4 + 1) * 512)
                    psq = ps_acc.tile([P, 512], F32, tag="acc")
                    for kk in range(8):
                        nc.tensor.matmul(
                            psq[:],
                            lhsT=wqT_sb[:, kk, p * 128 : (p + 1) * 128],
                            rhs=xT_sb[:, kk, tok],
                            start=(kk == 0),
                            stop=(kk == 7),
                        )
                    nc.vector.tensor_scalar_add(
                        q_sb[:, p, tok], psq[:], qb_sb[:, p : p + 1]
                    )
                    psk = ps_acc.tile([P, 512], F32, tag="acc")
                    for kk in range(8):
                        nc.tensor.matmul(
                            psk[:],
                            lhsT=wkT_sb[:, kk, p * 128 : (p + 1) * 128],
                            rhs=xT_sb[:, kk, tok],
                            start=(kk == 0),
                            stop=(kk == 7),
                        )
                    nc.vector.tensor_scalar_add(
                        k_sb[:, p, tok], psk[:], kb_sb[:, p : p + 1]
                    )

                for qtc in range(4):
                    qt = slice(qtc * 512, (qtc + 1) * 512)
                    blk = p * 4 + qtc
                    ps_a = ps_acc.tile([P, 512], F32, tag="acc")
                    ps_s = ps_misc.tile([P, 512], F32, tag="misc")
                    for kt in range(16):
                        kts = slice(kt * 128, (kt + 1) * 128)
                        if p == 0 and qtc == 0:
                            v_proj(kt)
                        sc = ps_sc.tile([P, 1024], F32, tag="sc")
                        # transposed scores for both heads of the pair
                        nc.tensor.matmul(
                            sc[:, 0:512],
                            lhsT=k_sb[0:64, p, kts],
                            rhs=q_sb[0:64, p, qt],
                            start=True,
                            stop=True,
                        )
                        nc.tensor.matmul(
                            sc[:, 512:1024],
                            lhsT=k_sb[64:128, p, kts],
                            rhs=q_sb[64:128, p, qt],
                            start=True,
                            stop=True,
                        )
                        pt = ptpool.tile([P, 1024], BF16, tag="pt")
                        nc.scalar.activation(pt[:], sc[:], Exp, scale=0.125)
                        # attended (both heads packed on output partitions)
                        nc.tensor.matmul(
                            ps_a[0:64, :],
                            lhsT=vT_sb[:, kt, p * 128 : p * 128 + 64],
                            rhs=pt[:, 0:512],
                            start=(kt == 0),
                            stop=(kt == 15),
                        )
                        nc.tensor.matmul(
                            ps_a[64:128, :],
                            lhsT=vT_sb[:, kt, p * 128 + 64 : p * 128 + 128],
                            rhs=pt[:, 512:1024],
                            start=(kt == 0),
                            stop=(kt == 15),
                            tile_position=(0, 64),
                        )
                        # denominators (row sums of exp) via M=1 matmuls
                        nc.tensor.matmul(
                            ps_s[0:1, :],
                            lhsT=onesk[:, 0:1],
                            rhs=pt[:, 0:512],
                            start=(kt == 0),
                            stop=(kt == 15),
                        )
                        nc.tensor.matmul(
                            ps_s[32:33, :],
                            lhsT=onesk[:, 0:1],
                            rhs=pt[:, 512:1024],
                            start=(kt == 0),
                            stop=(kt == 15),
                            tile_position=(0, 32),
                        )
                    nc.vector.tensor_copy(attU_sb[:, p, qt], ps_a[:])
                    nc.vector.tensor_copy(stage_sb[0:1, blk, :], ps_s[0:1, :])
                    nc.vector.tensor_copy(stage_sb[32:33, blk, :], ps_s[32:33, :])

            # ---- softmax normalization ----
            for blk in range(16):
                nc.sync.dma_start(
                    d32_sb[2 * blk : 2 * blk + 1, :], stage_sb[0:1, blk, :]
                )
                nc.sync.dma_start(
                    d32_sb[2 * blk + 1 : 2 * blk + 2, :], stage_sb[32:33, blk, :]
                )
            with nc.allow_low_precision(reason="softmax denom reciprocal in bf16"):
                nc.vector.reciprocal(r32_sb[:], d32_sb[:])
            for blk in range(16):
                nc.sync.dma_start(r2_sb[0:1, blk, :], r32_sb[2 * blk : 2 * blk + 1, :])
                nc.sync.dma_start(
                    r2_sb[1:2, blk, :], r32_sb[2 * blk + 1 : 2 * blk + 2, :]
                )
            for p in range(4):
                for qtc in range(4):
                    qt = slice(qtc * 512, (qtc + 1) * 512)
                    blk = p * 4 + qtc
                    ps_b = ps_misc.tile([P, 512], F32, tag="misc")
                    nc.tensor.matmul(
                        ps_b[:],
                        lhsT=sel2[0:2, :],
                        rhs=r2_sb[0:2, blk, :],
                        start=True,
                        stop=True,
                    )
                    nc.vector.tensor_mul(attU_sb[:, p, qt], attU_sb[:, p, qt], ps_b[:])

            # ---- dense projection (partial; host adds the other half + bias)
            for tt in range(16):
                tts = slice(tt * 128, (tt + 1) * 128)
                for oc in range(2):
                    ocs = slice(oc * 512, (oc + 1) * 512)
                    ps = ps_acc.tile([P, 512], F32, tag="acc")
                    for kk in range(4):
                        nc.tensor.matmul(
                            ps[:],
                            lhsT=attU_sb[:, kk, tts],
                            rhs=dwT_sb[:, kk, ocs],
                            start=(kk == 0),
                            stop=(kk == 3),
                        )
                    ot = evpool.tile([P, 512], F32, tag="out")
                    nc.vector.tensor_copy(ot[:], ps[:])
                    nc.sync.dma_start(outp[tts, ocs], ot[:])

        if loop_r:
            with tc.For_i(0, loop_r, 1):
                body()
        else:
            body()

    nc.compile()
    return nc


# ---------------------------------------------------------------------------
# PJRT runner (modeled on concourse.bass2jax.run_bass_via_pjrt, but caches the
# jitted executable so repeated calls don't retrace/recompile).
# ---------------------------------------------------------------------------
_CACHE = {}


def _make_runner(loop_r=None):
    import jax
    from jax.sharding import Mesh, PartitionSpec
    from jax.experimental.shard_map import shard_map

    from concourse import bass2jax
    from concourse import mybir as _mybir

    nc = _build_nc(loop_r=loop_r)
    bass2jax.install_neuronx_cc_hook()

    partition_name = nc.partition_id_tensor.name if nc.partition_id_tensor else None
    in_names, out_names, out_avals = [], [], []
    for alloc in nc.m.functions[0].allocations:
        if not isinstance(alloc, _mybir.MemoryLocationSet):
            continue
        name = alloc.memorylocations[0].name
        if alloc.kind == "ExternalInput":
            if name != partition_name:
                in_names.append(name)
        elif alloc.kind == "ExternalOutput":
            out_names.append(name)
            out_avals.append(
                jax.core.ShapedArray(
                    tuple(alloc.tensor_shape), _mybir.dt.np(alloc.dtype)
                )
            )
    n_params = len(in_names)
    all_in_names = list(in_names) + list(out_names)
    if partition_name is not None:
        all_in_names.append(partition_name)

    def _body(*args):
        operands = list(args)
        if partition_name is not None:
            operands.append(bass2jax.partition_id_tensor())
        outs = bass2jax._bass_exec_p.bind(
            *operands,
            out_avals=tuple(out_avals),
            in_names=tuple(all_in_names),
            out_names=tuple(out_names),
            lowering_input_output_aliases=(),
            sim_require_finite=True,
            sim_require_nnan=True,
            nc=nc,
        )
        return tuple(outs)

    devices = jax.devices()[:8]
    mesh = Mesh(np.asarray(devices), ("core",))
    in_specs = (PartitionSpec("core"),) * (n_params + len(out_names))
    out_specs = (PartitionSpec("core"),) * len(out_names)
    jitted = jax.jit(
        shard_map(
            _body, mesh=mesh, in_specs=in_specs, out_specs=out_specs, check_rep=False
        ),
        keep_unused=True,
    )
    zeros = [np.zeros((8 * av.shape[0], *av.shape[1:]), av.dtype) for av in out_avals]
    return (jitted, in_names, out_names, out_avals, zeros, mesh)


def _get_runner(loop_r=None):
    key = ("runner", loop_r)
    if key not in _CACHE:
        _CACHE[key] = _make_runner(loop_r)
    return _CACHE[key]


def _prep_core_inputs(x, wq_w, wq_b, wk_w, wk_b, wv_w, wv_b, dense_w):
    """Per-core host-side shard prep. Returns list of dicts (8 cores)."""
    maps = []
    for c in range(8):
        b, half = c // 2, c % 2
        f0 = half * HHALF
        fs = slice(f0, f0 + HHALF)
        maps.append(
            {
                "xT": np.ascontiguousarray(x[b].T).astype(NPBF16),
                "wqT": np.ascontiguousarray(wq_w[fs].T).astype(NPBF16),
                "wkT": np.ascontiguousarray(wk_w[fs].T).astype(NPBF16),
                "wvT": np.ascontiguousarray(wv_w[fs].T).astype(NPBF16),
                "dwT": np.ascontiguousarray(dense_w[:, fs].T).astype(NPBF16),
                "qb": np.ascontiguousarray(wq_b[fs].reshape(4, P).T.astype(np.float32)),
                "kb": np.ascontiguousarray(wk_b[fs].reshape(4, P).T.astype(np.float32)),
                "vb": np.broadcast_to(
                    wv_b[fs].reshape(1, HHALF).astype(NPBF16), (P, HHALF)
                ).copy(),
                "sel": _SEL2,
            }
        )
    return maps


def run_device(in_maps, time_iters=0, loop_r=None):
    """Run the SPMD kernel. Returns (per-core outp list, best wall ns or None)."""
    jitted, in_names, out_names, out_avals, zeros, mesh = _get_runner(loop_r)
    concat_in = [
        np.concatenate([in_maps[c][name] for c in range(8)], axis=0)
        for name in in_names
    ]
    args = concat_in + zeros
    outs = jitted(*args)
    outs = [np.asarray(o) for o in outs]
    best_ns = None
    if time_iters:
        import jax
        from jax.sharding import NamedSharding, PartitionSpec

        sh = NamedSharding(mesh, PartitionSpec("core"))
        dev_args = [jax.device_put(a, sh) for a in args]
        jax.block_until_ready(dev_args)
        times = []
        for _ in range(time_iters):
            t0 = time.perf_counter()
            o = jitted(*dev_args)
            jax.block_until_ready(o)
            times.append(time.perf_counter() - t0)
        best_ns = int(min(times) * 1e9)
    per_core = [
        {
            name: outs[i].reshape(8, *out_avals[i].shape)[c]
            for i, name in enumerate(out_names)
        }
        for c in range(8)
    ]
    return per_core, best_ns


def kernel(**inputs):
    x = np.asarray(inputs["x"], np.float32)
    args = {
        k: np.asarray(inputs[k], np.float32)
        for k in ["wq_w", "wq_b", "wk_w", "wk_b", "wv_w", "wv_b", "dense_w"]
    }
    in_maps = _prep_core_inputs(x, **args)
    per_core, _ = run_device(in_maps)
    dense_b = np.asarray(inputs["dense_b"], np.float32)
    out = np.empty((B, S, D), np.float32)
    for b in range(B):
        out[b] = per_core[2 * b]["outp"] + per_core[2 * b + 1]["outp"] + dense_b
    return out



# revision 8
# speedup vs baseline: 167.1786x; 167.1786x over previous
"""Trainium2 Bass kernel for MemoryEfficientMultiHeadAttention (8 NeuronCores).

Sharding: hybrid data/tensor parallel. Core c handles batch b = c//2 and head
group half = c%2 (8 of 16 heads, i.e. 512 of 1024 qkv features). Each core:
  q,k  = (x_b @ w.T + b) in [feat, tok] layout (feat on partitions)
  vT   = (x_b @ wv.T + b) in [tok, feat] layout, with a ones column appended
         per head so the attended matmul also produces the softmax denominator
  per head pair: scoresT = k_h.T @ q_h   (transposed scores, [kt, qt]; the two
                 heads run as row-tiled concurrent matmuls)
            PT = exp(scoresT / 8)        (no max-subtraction: scores are O(1))
            attU[0:64] / denom[64] += [vT_h | 1].T @ PT   (M=65, over kt tiles)
  attS = attU * (1/denom)  broadcast to partitions via gpsimd
  outp = attS.T @ dense_w_slice.T         (partial over this core's 512 feats)
Host: out[b] = outp[2b] + outp[2b+1] + dense_b.

The schedule targets the ACT engine's exp throughput (the roofline for this
decomposition): scores for iteration kt+1 are issued before the attended
matmuls of iteration kt so the PE never head-blocks the exp stream, and the
q/k projections of the next head pair plus the dense output matmuls are
drip-fed into the PE's idle slots (one ~4-matmul burst per odd kt iteration).

All matmuls run in bf16 (1 cycle/row on TRN2 PE; fp32 is 4 cycles/row) with
fp32 PSUM accumulation.
"""

import sys
import time
from contextlib import ExitStack

import numpy as np

try:
    import concourse.bass as bass  # noqa: F401
except ImportError:  # pragma: no cover
    sys.path.insert(0, "/opt/trn_rl_repo")

import ml_dtypes

import concourse.bacc as bacc
import concourse.mybir as mybir
import concourse.tile as tile

P = 128
BF16 = mybir.dt.bfloat16
F32 = mybir.dt.float32
NPBF16 = ml_dtypes.bfloat16

B, S, D = 4, 2048, 1024
HHALF = 512  # features per core (8 heads x 64)


def _build_nc(loop_r=None):
    nc = bacc.Bacc()

    xT = nc.dram_tensor("xT", [D, S], BF16, kind="ExternalInput")
    wqT = nc.dram_tensor("wqT", [D, HHALF], BF16, kind="ExternalInput")
    wkT = nc.dram_tensor("wkT", [D, HHALF], BF16, kind="ExternalInput")
    wvT = nc.dram_tensor("wvT", [D, HHALF], BF16, kind="ExternalInput")
    dwT = nc.dram_tensor("dwT", [HHALF, D], BF16, kind="ExternalInput")
    qb = nc.dram_tensor("qb", [P, 4], F32, kind="ExternalInput")
    kb = nc.dram_tensor("kb", [P, 4], F32, kind="ExternalInput")
    vb = nc.dram_tensor("vb", [P, 8, 64], BF16, kind="ExternalInput")
    outp = nc.dram_tensor("outp", [S, D], F32, kind="ExternalOutput")

    Exp = mybir.ActivationFunctionType.Exp

    with tile.TileContext(nc) as tc, ExitStack() as ctx:
        wpool = ctx.enter_context(tc.tile_pool(name="weights", bufs=1))
        spool = ctx.enter_context(tc.tile_pool(name="state", bufs=1))
        ptpool = ctx.enter_context(tc.tile_pool(name="pt", bufs=3))
        dpool = ctx.enter_context(tc.tile_pool(name="den", bufs=4))
        rpool = ctx.enter_context(tc.tile_pool(name="rec", bufs=8))
        scpool = ctx.enter_context(tc.tile_pool(name="scale", bufs=6))
        evpool = ctx.enter_context(tc.tile_pool(name="evac", bufs=4))
        ps_sc = ctx.enter_context(tc.tile_pool(name="pssc", bufs=2, space="PSUM"))
        ps_sm = ctx.enter_context(tc.tile_pool(name="pssm", bufs=4, space="PSUM"))

        # ---- persistent SBUF state (loaded once) ----
        xT_sb = wpool.tile([P, 8, S], BF16)
        xT_r = xT.rearrange("(o p) t -> p o t", p=P)
        for kk in range(8):  # split per chunk so early chunks land early
            nc.sync.dma_start(xT_sb[:, kk, :], xT_r[:, kk, :])
        wqT_sb = wpool.tile([P, 8, HHALF], BF16)
        nc.sync.dma_start(wqT_sb[:], wqT.rearrange("(o p) f -> p o f", p=P))
        wkT_sb = wpool.tile([P, 8, HHALF], BF16)
        nc.sync.dma_start(wkT_sb[:], wkT.rearrange("(o p) f -> p o f", p=P))
        wvT_sb = wpool.tile([P, 8, HHALF], BF16)
        nc.sync.dma_start(wvT_sb[:], wvT.rearrange("(o p) f -> p o f", p=P))
        dwT_sb = wpool.tile([P, 4, D], BF16)
        nc.sync.dma_start(dwT_sb[:], dwT.rearrange("(o p) f -> p o f", p=P))
        qb_sb = wpool.tile([P, 4], F32)
        nc.sync.dma_start(qb_sb[:], qb[:])
        kb_sb = wpool.tile([P, 4], F32)
        nc.sync.dma_start(kb_sb[:], kb[:])
        vb_sb = wpool.tile([P, 8, 64], BF16)
        nc.sync.dma_start(vb_sb[:], vb[:])

        # per-(pair,t4) projection tiles; per-strip vT tiles (fine-grained
        # tiles keep the scheduler's dependency tracking precise)
        q_sb = [[spool.tile([P, 512], BF16, name=f"q_{p}_{t}") for t in range(4)] for p in range(4)]
        k_sb = [[spool.tile([P, 512], BF16, name=f"k_{p}_{t}") for t in range(4)] for p in range(4)]
        # vT strip: 8 heads x (64 feats + ones col). memset once: the ones
        # columns (index 64) are never overwritten by the projection.
        vT_sb = [spool.tile([P, 8, 65], BF16, name=f"vT_{t}") for t in range(16)]
        for t in range(16):
            nc.vector.memset(vT_sb[t][:], 1.0)
        attU_sb = [spool.tile([P, 4, 512], BF16, name=f"attU_{q}") for q in range(4)]  # per qtc

        def v_unit(t):
            psv = ps_sm.tile([P, 8, 64], F32, tag="sm")
            for kk in range(8):
                nc.tensor.matmul(
                    psv[:],
                    lhsT=xT_sb[:, kk, t * 128 : (t + 1) * 128],
                    rhs=wvT_sb[:, kk, :],
                    start=(kk == 0),
                    stop=(kk == 7),
                )
            nc.vector.tensor_add(vT_sb[t][:, :, 0:64], psv[:], vb_sb[:])

        proj_state = {}

        def proj_half(p, j, half):
            """4 matmuls of projection unit j for pair p; evac on 2nd half."""
            t4 = j // 2
            isq = j % 2 == 0
            w_sb = wqT_sb if isq else wkT_sb
            if half == 0:
                proj_state[(p, j)] = ps_sm.tile(
                    [P, 512], F32, tag="sm", name=f"psproj_{p}_{j}"
                )
            ps = proj_state[(p, j)]
            for kk in range(half * 4, half * 4 + 4):
                nc.tensor.matmul(
                    ps[:],
                    lhsT=w_sb[:, kk, p * 128 : (p + 1) * 128],
                    rhs=xT_sb[:, kk, t4 * 512 : (t4 + 1) * 512],
                    start=(kk == 0),
                    stop=(kk == 7),
                )
            if half == 1:
                dst = q_sb[p][t4] if isq else k_sb[p][t4]
                b_sb = qb_sb if isq else kb_sb
                nc.vector.tensor_scalar_add(dst[:], ps[:], b_sb[:, p : p + 1])
                del proj_state[(p, j)]

        def dense_half(tt, oc):
            """dense output tile (tt, oc): 4 matmuls + evac + DMA."""
            qtc, ts = tt // 4, (tt % 4) * 128
            ps = ps_sm.tile([P, 512], F32, tag="sm")
            for kk in range(4):
                nc.tensor.matmul(
                    ps[:],
                    lhsT=attU_sb[qtc][:, kk, ts : ts + 128],
                    rhs=dwT_sb[:, kk, oc * 512 : (oc + 1) * 512],
                    start=(kk == 0),
                    stop=(kk == 3),
                )
            ot = evpool.tile([P, 512], F32, tag="out")
            nc.vector.tensor_copy(ot[:], ps[:])
            nc.sync.dma_start(
                outp[tt * 128 : (tt + 1) * 128, oc * 512 : (oc + 1) * 512], ot[:]
            )

        def body():
            # pair-0 projections run up front (everything else interleaves)
            for j in range(8):
                proj_half(0, j, 0)
                proj_half(0, j, 1)

            for p in range(4):
                if p == 0:
                    v_unit(0)  # strips 1..15 are produced inside blk(0,0)
                # side-work consumed one closure per odd kt iteration
                side = []
                if p < 3:
                    for j in range(8):
                        side.append(lambda p=p, j=j: proj_half(p + 1, j, 0))
                        side.append(lambda p=p, j=j: proj_half(p + 1, j, 1))

                for qtc in range(4):
                    if p == 3 and qtc > 0:
                        for tt in range(4 * (qtc - 1), 4 * qtc):
                            side.append(lambda tt=tt: dense_half(tt, 0))
                            side.append(lambda tt=tt: dense_half(tt, 1))
                    qt = slice(qtc * 512, (qtc + 1) * 512)
                    ps_aA = ps_sm.tile([P, 512], F32, tag="sm")
                    ps_aB = ps_sm.tile([P, 512], F32, tag="sm")

                    def scores(kt):
                        sc = ps_sc.tile([P, 1024], F32, tag="sc")
                        kts = slice((kt % 4) * 128, (kt % 4) * 128 + 128)
                        nc.tensor.matmul(
                            sc[:, 0:512],
                            lhsT=k_sb[p][kt // 4][0:64, kts],
                            rhs=q_sb[p][qtc][0:64, :],
                            start=True,
                            stop=True,
                        )
                        nc.tensor.matmul(
                            sc[:, 512:1024],
                            lhsT=k_sb[p][kt // 4][64:128, kts],
                            rhs=q_sb[p][qtc][64:128, :],
                            start=True,
                            stop=True,
                        )
                        return sc

                    sc_cur = scores(0)
                    for kt in range(16):
                        if p == 0 and qtc == 0:
                            if kt < 15:
                                v_unit(kt + 1)  # strip kt+1 ready before its use
                        elif kt % 2 == 1 and side and (p < 3 or kt >= 5):
                            side.pop(0)()
                        pt = ptpool.tile([P, 1024], BF16, tag="pt")
                        nc.scalar.activation(pt[:], sc_cur[:], Exp, scale=0.125)
                        if kt < 15:
                            sc_cur = scores(kt + 1)
                        nc.tensor.matmul(
                            ps_aA[0:65, :],
                            lhsT=vT_sb[kt][:, 2 * p, :],
                            rhs=pt[:, 0:512],
                            start=(kt == 0),
                            stop=(kt == 15),
                        )
                        nc.tensor.matmul(
                            ps_aB[0:65, :],
                            lhsT=vT_sb[kt][:, 2 * p + 1, :],
                            rhs=pt[:, 512:1024],
                            start=(kt == 0),
                            stop=(kt == 15),
                        )

                    # Evacuate attended + denominator partition-aligned (the
                    # DVE has no cross-lane path: partition moves must go via
                    # SBUF->SBUF DMA), then normalize.
                    stA = dpool.tile([65, 512], F32, tag="d")
                    stB = dpool.tile([65, 512], F32, tag="d")
                    nc.vector.tensor_copy(stA[:], ps_aA[0:65, :])
                    nc.vector.tensor_copy(stB[:], ps_aB[0:65, :])
                    dA = rpool.tile([1, 512], F32, tag="r")
                    dB = rpool.tile([1, 512], F32, tag="r")
                    nc.sync.dma_start(dA[:], stA[64:65, :])
                    nc.sync.dma_start(dB[:], stB[64:65, :])
                    rA = rpool.tile([1, 512], F32, tag="r")
                    rB = rpool.tile([1, 512], F32, tag="r")
                    nc.vector.reciprocal_approx_fast(rA[:], dA[:])
                    nc.vector.reciprocal_approx_fast(rB[:], dB[:])
                    scA = scpool.tile([64, 512], F32, tag="s")
                    scB = scpool.tile([64, 512], F32, tag="s")
                    nc.gpsimd.partition_broadcast(scA[:, :], rA[0:1, :], 64)
                    nc.gpsimd.partition_broadcast(scB[:, :], rB[0:1, :], 64)
                    nc.vector.tensor_mul(
                        attU_sb[qtc][0:64, p, :], stA[0:64, :], scA[:, :]
                    )
                    attBn = scpool.tile([64, 512], BF16, tag="s")
                    nc.vector.tensor_mul(attBn[:, :], stB[0:64, :], scB[:, :])
                    nc.sync.dma_start(attU_sb[qtc][64:128, p, :], attBn[:, :])

                # flush any leftover side work before this pair's last block
                while side:
                    side.pop(0)()

            # dense tail: last token quarter
            for tt in range(12, 16):
                dense_half(tt, 0)
                dense_half(tt, 1)

        if loop_r:
            with tc.For_i(0, loop_r, 1):
                body()
        else:
            body()

    nc.compile()
    return nc


# ---------------------------------------------------------------------------
# PJRT runner (modeled on concourse.bass2jax.run_bass_via_pjrt, but caches the
# jitted executable so repeated calls don't retrace/recompile).
# ---------------------------------------------------------------------------
_CACHE = {}


def _make_runner(loop_r=None):
    import jax
    from jax.sharding import Mesh, PartitionSpec
    from jax.experimental.shard_map import shard_map

    from concourse import bass2jax
    from concourse import mybir as _mybir

    nc = _build_nc(loop_r=loop_r)
    bass2jax.install_neuronx_cc_hook()

    partition_name = nc.partition_id_tensor.name if nc.partition_id_tensor else None
    in_names, out_names, out_avals = [], [], []
    for alloc in nc.m.functions[0].allocations:
        if not isinstance(alloc, _mybir.MemoryLocationSet):
            continue
        name = alloc.memorylocations[0].name
        if alloc.kind == "ExternalInput":
            if name != partition_name:
                in_names.append(name)
        elif alloc.kind == "ExternalOutput":
            out_names.append(name)
            out_avals.append(
                jax.core.ShapedArray(
                    tuple(alloc.tensor_shape), _mybir.dt.np(alloc.dtype)
                )
            )
    n_params = len(in_names)
    all_in_names = list(in_names) + list(out_names)
    if partition_name is not None:
        all_in_names.append(partition_name)

    def _body(*args):
        operands = list(args)
        if partition_name is not None:
            operands.append(bass2jax.partition_id_tensor())
        outs = bass2jax._bass_exec_p.bind(
            *operands,
            out_avals=tuple(out_avals),
            in_names=tuple(all_in_names),
            out_names=tuple(out_names),
            lowering_input_output_aliases=(),
            sim_require_finite=True,
            sim_require_nnan=True,
            nc=nc,
        )
        return tuple(outs)

    devices = jax.devices()[:8]
    mesh = Mesh(np.asarray(devices), ("core",))
    in_specs = (PartitionSpec("core"),) * (n_params + len(out_names))
    out_specs = (PartitionSpec("core"),) * len(out_names)
    jitted = jax.jit(
        shard_map(
            _body, mesh=mesh, in_specs=in_specs, out_specs=out_specs, check_rep=False
        ),
        keep_unused=True,
    )
    zeros = [np.zeros((8 * av.shape[0], *av.shape[1:]), av.dtype) for av in out_avals]
    return (jitted, in_names, out_names, out_avals, zeros, mesh)


def _get_runner(loop_r=None):
    key = ("runner", loop_r)
    if key not in _CACHE:
        _CACHE[key] = _make_runner(loop_r)
    return _CACHE[key]


def _prep_core_inputs(x, wq_w, wq_b, wk_w, wk_b, wv_w, wv_b, dense_w):
    """Per-core host-side shard prep. Returns list of dicts (8 cores)."""
    maps = []
    for c in range(8):
        b, half = c // 2, c % 2
        f0 = half * HHALF
        fs = slice(f0, f0 + HHALF)
        maps.append(
            {
                "xT": np.ascontiguousarray(x[b].T).astype(NPBF16),
                "wqT": np.ascontiguousarray(wq_w[fs].T).astype(NPBF16),
                "wkT": np.ascontiguousarray(wk_w[fs].T).astype(NPBF16),
                "wvT": np.ascontiguousarray(wv_w[fs].T).astype(NPBF16),
                "dwT": np.ascontiguousarray(dense_w[:, fs].T).astype(NPBF16),
                "qb": np.ascontiguousarray(wq_b[fs].reshape(4, P).T.astype(np.float32)),
                "kb": np.ascontiguousarray(wk_b[fs].reshape(4, P).T.astype(np.float32)),
                "vb": np.broadcast_to(
                    wv_b[fs].reshape(1, 8, 64).astype(NPBF16), (P, 8, 64)
                ).copy(),
            }
        )
    return maps


def run_device(in_maps, time_iters=0, loop_r=None):
    """Run the SPMD kernel. Returns (per-core outp list, best wall ns or None)."""
    jitted, in_names, out_names, out_avals, zeros, mesh = _get_runner(loop_r)
    concat_in = [
        np.concatenate([in_maps[c][name] for c in range(8)], axis=0)
        for name in in_names
    ]
    args = concat_in + zeros
    outs = jitted(*args)
    outs = [np.asarray(o) for o in outs]
    best_ns = None
    if time_iters:
        import jax
        from jax.sharding import NamedSharding, PartitionSpec

        sh = NamedSharding(mesh, PartitionSpec("core"))
        dev_args = [jax.device_put(a, sh) for a in args]
        jax.block_until_ready(dev_args)
        times = []
        for _ in range(time_iters):
            t0 = time.perf_counter()
            o = jitted(*dev_args)
            jax.block_until_ready(o)
            times.append(time.perf_counter() - t0)
        best_ns = int(min(times) * 1e9)
    per_core = [
        {
            name: outs[i].reshape(8, *out_avals[i].shape)[c]
            for i, name in enumerate(out_names)
        }
        for c in range(8)
    ]
    return per_core, best_ns


def kernel(**inputs):
    x = np.asarray(inputs["x"], np.float32)
    args = {
        k: np.asarray(inputs[k], np.float32)
        for k in ["wq_w", "wq_b", "wk_w", "wk_b", "wv_w", "wv_b", "dense_w"]
    }
    in_maps = _prep_core_inputs(x, **args)
    per_core, _ = run_device(in_maps)
    dense_b = np.asarray(inputs["dense_b"], np.float32)
    out = np.empty((B, S, D), np.float32)
    for b in range(B):
        out[b] = per_core[2 * b]["outp"] + per_core[2 * b + 1]["outp"] + dense_b
    return out


# revision 11
# speedup vs baseline: 252.5084x; 1.5104x over previous
"""Trainium2 Bass kernel for MemoryEfficientMultiHeadAttention (8 NeuronCores).

Sharding: hybrid data/tensor parallel. Core c handles batch b = c//2 and head
group half = c%2 (8 of 16 heads, i.e. 512 of 1024 qkv features). Each core:
  q,k  = (x_b @ w.T + b) in [feat, tok] layout (feat on partitions)
  vT   = (x_b @ wv.T + b) in [tok, feat] layout, with a ones column appended
         per head so the attended matmul also produces the softmax denominator
  per head pair: scoresT = k_h.T @ q_h   (transposed scores, [kt, qt]; the two
                 heads run as row-tiled concurrent matmuls)
            PT = exp(scoresT / 8)        (no max-subtraction: scores are O(1))
            attU[0:64] / denom[64] += [vT_h | 1].T @ PT   (M=65, over kt tiles)
  attS = attU * (1/denom)  broadcast to partitions via gpsimd
  outp = attS.T @ dense_w_slice.T         (partial over this core's 512 feats)
Host: out[b] = outp[2b] + outp[2b+1] + dense_b.

The schedule targets the ACT engine's exp throughput (the roofline for this
decomposition): scores for iteration kt+1 are issued before the attended
matmuls of iteration kt so the PE never head-blocks the exp stream, and the
q/k projections of the next head pair plus the dense output matmuls are
drip-fed into the PE's idle slots (one ~4-matmul burst per odd kt iteration).

All matmuls run in bf16 (1 cycle/row on TRN2 PE; fp32 is 4 cycles/row) with
fp32 PSUM accumulation.
"""

import sys
import time
from contextlib import ExitStack

import numpy as np

try:
    import concourse.bass as bass  # noqa: F401
except ImportError:  # pragma: no cover
    sys.path.insert(0, "/opt/trn_rl_repo")

import ml_dtypes

import concourse.bacc as bacc
import concourse.mybir as mybir
import concourse.tile as tile

P = 128
BF16 = mybir.dt.bfloat16
FP8 = mybir.dt.float8e4
F32 = mybir.dt.float32
NPBF16 = ml_dtypes.bfloat16

# exp output / V dtype. fp8e4m3 would halve ACT SBUF-write and PE rhs-read
# traffic, but measures ~2e-2 rel err (attention outputs are themselves
# softmax averages, so quantization noise is not averaged down) — keep bf16.
PT_DT = BF16

B, S, D = 4, 2048, 1024
HHALF = 512  # features per core (8 heads x 64)


def _build_nc(loop_r=None):
    nc = bacc.Bacc()

    xT = nc.dram_tensor("xT", [D, S], BF16, kind="ExternalInput")
    wqT = nc.dram_tensor("wqT", [D, HHALF], BF16, kind="ExternalInput")
    wkT = nc.dram_tensor("wkT", [D, HHALF], BF16, kind="ExternalInput")
    wvT = nc.dram_tensor("wvT", [D, HHALF], BF16, kind="ExternalInput")
    dwT = nc.dram_tensor("dwT", [HHALF, D], BF16, kind="ExternalInput")
    qb = nc.dram_tensor("qb", [P, 4], F32, kind="ExternalInput")
    kb = nc.dram_tensor("kb", [P, 4], F32, kind="ExternalInput")
    vb = nc.dram_tensor("vb", [P, 8, 64], BF16, kind="ExternalInput")
    outp = nc.dram_tensor("outp", [S, D], F32, kind="ExternalOutput")

    Exp = mybir.ActivationFunctionType.Exp

    with tile.TileContext(nc) as tc, ExitStack() as ctx:
        wpool = ctx.enter_context(tc.tile_pool(name="weights", bufs=1))
        spool = ctx.enter_context(tc.tile_pool(name="state", bufs=1))
        ptpool = ctx.enter_context(tc.tile_pool(name="pt", bufs=4))
        dpool = ctx.enter_context(tc.tile_pool(name="den", bufs=4))
        rpool = ctx.enter_context(tc.tile_pool(name="rec", bufs=8))
        scpool = ctx.enter_context(tc.tile_pool(name="scale", bufs=6))
        evpool = ctx.enter_context(tc.tile_pool(name="evac", bufs=4))
        ps_sc = ctx.enter_context(tc.tile_pool(name="pssc", bufs=2, space="PSUM"))
        ps_sm = ctx.enter_context(tc.tile_pool(name="pssm", bufs=4, space="PSUM"))

        # ---- persistent SBUF state (loaded once) ----
        xT_sb = wpool.tile([P, 8, S], BF16)
        xT_r = xT.rearrange("(o p) t -> p o t", p=P)
        for kk in range(8):  # split per chunk so early chunks land early
            nc.sync.dma_start(xT_sb[:, kk, :], xT_r[:, kk, :])
        wqT_sb = wpool.tile([P, 8, HHALF], BF16)
        nc.sync.dma_start(wqT_sb[:], wqT.rearrange("(o p) f -> p o f", p=P))
        wkT_sb = wpool.tile([P, 8, HHALF], BF16)
        nc.sync.dma_start(wkT_sb[:], wkT.rearrange("(o p) f -> p o f", p=P))
        wvT_sb = wpool.tile([P, 8, HHALF], BF16)
        nc.sync.dma_start(wvT_sb[:], wvT.rearrange("(o p) f -> p o f", p=P))
        dwT_sb = wpool.tile([P, 4, D], BF16)
        nc.sync.dma_start(dwT_sb[:], dwT.rearrange("(o p) f -> p o f", p=P))
        qb_sb = wpool.tile([P, 4], F32)
        nc.sync.dma_start(qb_sb[:], qb[:])
        kb_sb = wpool.tile([P, 4], F32)
        nc.sync.dma_start(kb_sb[:], kb[:])
        vb_sb = wpool.tile([P, 8, 64], BF16)
        nc.sync.dma_start(vb_sb[:], vb[:])

        # per-(pair,t4) projection tiles; per-strip vT tiles (fine-grained
        # tiles keep the scheduler's dependency tracking precise)
        q_sb = [[spool.tile([P, 512], BF16, name=f"q_{p}_{t}") for t in range(4)] for p in range(4)]
        k_sb = [[spool.tile([P, 512], BF16, name=f"k_{p}_{t}") for t in range(4)] for p in range(4)]
        # vT strip: 8 heads x (64 feats + ones col). memset once: the ones
        # columns (index 64) are never overwritten by the projection.
        vT_sb = [spool.tile([P, 8, 65], PT_DT, name=f"vT_{t}") for t in range(16)]
        for t in range(16):
            nc.vector.memset(vT_sb[t][:], 1.0)
        attU_sb = [spool.tile([P, 4, 512], BF16, name=f"attU_{q}") for q in range(4)]  # per qtc

        def v_unit(t):
            psv = ps_sm.tile([P, 8, 64], F32, tag="sm")
            for kk in range(8):
                nc.tensor.matmul(
                    psv[:],
                    lhsT=xT_sb[:, kk, t * 128 : (t + 1) * 128],
                    rhs=wvT_sb[:, kk, :],
                    start=(kk == 0),
                    stop=(kk == 7),
                )
            nc.vector.tensor_add(vT_sb[t][:, :, 0:64], psv[:], vb_sb[:])

        proj_state = {}

        def proj_half(p, j, half):
            """4 matmuls of projection unit j for pair p; evac on 2nd half."""
            t4 = j // 2
            isq = j % 2 == 0
            w_sb = wqT_sb if isq else wkT_sb
            if half == 0:
                proj_state[(p, j)] = ps_sm.tile(
                    [P, 512], F32, tag="sm", name=f"psproj_{p}_{j}"
                )
            ps = proj_state[(p, j)]
            for kk in range(half * 4, half * 4 + 4):
                nc.tensor.matmul(
                    ps[:],
                    lhsT=w_sb[:, kk, p * 128 : (p + 1) * 128],
                    rhs=xT_sb[:, kk, t4 * 512 : (t4 + 1) * 512],
                    start=(kk == 0),
                    stop=(kk == 7),
                )
            if half == 1:
                dst = q_sb[p][t4] if isq else k_sb[p][t4]
                b_sb = qb_sb if isq else kb_sb
                nc.vector.tensor_scalar_add(dst[:], ps[:], b_sb[:, p : p + 1])
                del proj_state[(p, j)]

        def dense_half(tt, oc):
            """dense output tile (tt, oc): 4 matmuls + evac + DMA."""
            qtc, ts = tt // 4, (tt % 4) * 128
            ps = ps_sm.tile([P, 512], F32, tag="sm")
            for kk in range(4):
                nc.tensor.matmul(
                    ps[:],
                    lhsT=attU_sb[qtc][:, kk, ts : ts + 128],
                    rhs=dwT_sb[:, kk, oc * 512 : (oc + 1) * 512],
                    start=(kk == 0),
                    stop=(kk == 3),
                )
            ot = evpool.tile([P, 512], F32, tag="out")
            nc.vector.tensor_copy(ot[:], ps[:])
            nc.sync.dma_start(
                outp[tt * 128 : (tt + 1) * 128, oc * 512 : (oc + 1) * 512], ot[:]
            )

        def body():
            # minimal startup prefix: q/k for (pair 0, first token quarter)
            # and V strip 0; the rest is produced just in time inside the
            # first attention block.
            for j in (0, 1):
                proj_half(0, j, 0)
                proj_half(0, j, 1)
            v_unit(0)

            for p in range(4):
                # side-work consumed one closure per odd kt iteration
                side = []
                if p == 0:
                    for j in (4, 6):  # q units for t4=2,3 (t4=1 in blk(0,0))
                        side.append(lambda j=j: proj_half(0, j, 0))
                        side.append(lambda j=j: proj_half(0, j, 1))
                if p < 3:
                    for j in range(8):
                        side.append(lambda p=p, j=j: proj_half(p + 1, j, 0))
                        side.append(lambda p=p, j=j: proj_half(p + 1, j, 1))

                for qtc in range(4):
                    if p == 3 and qtc > 0:
                        for tt in range(4 * (qtc - 1), 4 * qtc):
                            side.append(lambda tt=tt: dense_half(tt, 0))
                            side.append(lambda tt=tt: dense_half(tt, 1))
                    qt = slice(qtc * 512, (qtc + 1) * 512)
                    ps_aA = ps_sm.tile([P, 512], F32, tag="sm")
                    ps_aB = ps_sm.tile([P, 512], F32, tag="sm")

                    def scores(kt):
                        sc = ps_sc.tile([P, 1024], F32, tag="sc")
                        kts = slice((kt % 4) * 128, (kt % 4) * 128 + 128)
                        nc.tensor.matmul(
                            sc[:, 0:512],
                            lhsT=k_sb[p][kt // 4][0:64, kts],
                            rhs=q_sb[p][qtc][0:64, :],
                            start=True,
                            stop=True,
                        )
                        nc.tensor.matmul(
                            sc[:, 512:1024],
                            lhsT=k_sb[p][kt // 4][64:128, kts],
                            rhs=q_sb[p][qtc][64:128, :],
                            start=True,
                            stop=True,
                        )
                        return sc

                    sc_cur = scores(0)
                    for kt in range(16):
                        if p == 0 and qtc == 0:
                            if kt < 15:
                                v_unit(kt + 1)  # strip kt+1 ready before its use
                            if kt in (1, 5, 9):  # k units t4=1..3, just in time
                                j = {1: 3, 5: 5, 9: 7}[kt]
                                proj_half(0, j, 0)
                                proj_half(0, j, 1)
                            elif kt in (11, 13):  # q unit t4=1 before blk(0,1)
                                proj_half(0, 2, kt == 13)
                        elif kt % 2 == 1 and side and (p < 3 or kt >= 5):
                            side.pop(0)()
                        pt = ptpool.tile([P, 1024], PT_DT, tag="pt")
                        nc.scalar.activation(pt[:], sc_cur[:], Exp, scale=0.125)
                        if kt < 15:
                            sc_cur = scores(kt + 1)
                        nc.tensor.matmul(
                            ps_aA[0:65, :],
                            lhsT=vT_sb[kt][:, 2 * p, :],
                            rhs=pt[:, 0:512],
                            start=(kt == 0),
                            stop=(kt == 15),
                        )
                        nc.tensor.matmul(
                            ps_aB[0:65, :],
                            lhsT=vT_sb[kt][:, 2 * p + 1, :],
                            rhs=pt[:, 512:1024],
                            start=(kt == 0),
                            stop=(kt == 15),
                        )

                    # Evacuate attended + denominator partition-aligned (the
                    # DVE has no cross-lane path: partition moves must go via
                    # SBUF->SBUF DMA), then normalize.
                    stA = dpool.tile([65, 512], F32, tag="d")
                    stB = dpool.tile([65, 512], F32, tag="d")
                    nc.vector.tensor_copy(stA[:], ps_aA[0:65, :])
                    nc.vector.tensor_copy(stB[:], ps_aB[0:65, :])
                    dA = rpool.tile([1, 512], F32, tag="r")
                    dB = rpool.tile([1, 512], F32, tag="r")
                    nc.sync.dma_start(dA[:], stA[64:65, :])
                    nc.sync.dma_start(dB[:], stB[64:65, :])
                    rA = rpool.tile([1, 512], F32, tag="r")
                    rB = rpool.tile([1, 512], F32, tag="r")
                    nc.vector.reciprocal_approx_fast(rA[:], dA[:])
                    nc.vector.reciprocal_approx_fast(rB[:], dB[:])
                    scA = scpool.tile([64, 512], F32, tag="s")
                    scB = scpool.tile([64, 512], F32, tag="s")
                    nc.gpsimd.partition_broadcast(scA[:, :], rA[0:1, :], 64)
                    nc.gpsimd.partition_broadcast(scB[:, :], rB[0:1, :], 64)
                    nc.vector.tensor_mul(
                        attU_sb[qtc][0:64, p, :], stA[0:64, :], scA[:, :]
                    )
                    attBn = scpool.tile([64, 512], BF16, tag="s")
                    nc.vector.tensor_mul(attBn[:, :], stB[0:64, :], scB[:, :])
                    nc.sync.dma_start(attU_sb[qtc][64:128, p, :], attBn[:, :])

                # flush any leftover side work before this pair's last block
                while side:
                    side.pop(0)()

            # dense tail: last token quarter
            for tt in range(12, 16):
                dense_half(tt, 0)
                dense_half(tt, 1)

        if loop_r:
            with tc.For_i(0, loop_r, 1):
                body()
        else:
            body()

    nc.compile()
    return nc


# ---------------------------------------------------------------------------
# PJRT runner (modeled on concourse.bass2jax.run_bass_via_pjrt, but caches the
# jitted executable so repeated calls don't retrace/recompile).
# ---------------------------------------------------------------------------
_CACHE = {}


def _make_runner(loop_r=None):
    import jax
    from jax.sharding import Mesh, PartitionSpec
    from jax.experimental.shard_map import shard_map

    from concourse import bass2jax
    from concourse import mybir as _mybir

    nc = _build_nc(loop_r=loop_r)
    bass2jax.install_neuronx_cc_hook()

    partition_name = nc.partition_id_tensor.name if nc.partition_id_tensor else None
    in_names, out_names, out_avals = [], [], []
    for alloc in nc.m.functions[0].allocations:
        if not isinstance(alloc, _mybir.MemoryLocationSet):
            continue
        name = alloc.memorylocations[0].name
        if alloc.kind == "ExternalInput":
            if name != partition_name:
                in_names.append(name)
        elif alloc.kind == "ExternalOutput":
            out_names.append(name)
            out_avals.append(
                jax.core.ShapedArray(
                    tuple(alloc.tensor_shape), _mybir.dt.np(alloc.dtype)
                )
            )
    n_params = len(in_names)
    all_in_names = list(in_names) + list(out_names)
    if partition_name is not None:
        all_in_names.append(partition_name)

    def _body(*args):
        operands = list(args)
        if partition_name is not None:
            operands.append(bass2jax.partition_id_tensor())
        outs = bass2jax._bass_exec_p.bind(
            *operands,
            out_avals=tuple(out_avals),
            in_names=tuple(all_in_names),
            out_names=tuple(out_names),
            lowering_input_output_aliases=(),
            sim_require_finite=True,
            sim_require_nnan=True,
            nc=nc,
        )
        return tuple(outs)

    devices = jax.devices()[:8]
    mesh = Mesh(np.asarray(devices), ("core",))
    in_specs = (PartitionSpec("core"),) * (n_params + len(out_names))
    out_specs = (PartitionSpec("core"),) * len(out_names)
    jitted = jax.jit(
        shard_map(
            _body, mesh=mesh, in_specs=in_specs, out_specs=out_specs, check_rep=False
        ),
        keep_unused=True,
    )
    zeros = [np.zeros((8 * av.shape[0], *av.shape[1:]), av.dtype) for av in out_avals]
    return (jitted, in_names, out_names, out_avals, zeros, mesh)


def _get_runner(loop_r=None):
    key = ("runner", loop_r)
    if key not in _CACHE:
        _CACHE[key] = _make_runner(loop_r)
    return _CACHE[key]


def _prep_core_inputs(x, wq_w, wq_b, wk_w, wk_b, wv_w, wv_b, dense_w):
    """Per-core host-side shard prep. Returns list of dicts (8 cores)."""
    maps = []
    for c in range(8):
        b, half = c // 2, c % 2
        f0 = half * HHALF
        fs = slice(f0, f0 + HHALF)
        maps.append(
            {
                "xT": np.ascontiguousarray(x[b].T).astype(NPBF16),
                "wqT": np.ascontiguousarray(wq_w[fs].T).astype(NPBF16),
                "wkT": np.ascontiguousarray(wk_w[fs].T).astype(NPBF16),
                "wvT": np.ascontiguousarray(wv_w[fs].T).astype(NPBF16),
                "dwT": np.ascontiguousarray(dense_w[:, fs].T).astype(NPBF16),
                "qb": np.ascontiguousarray(wq_b[fs].reshape(4, P).T.astype(np.float32)),
                "kb": np.ascontiguousarray(wk_b[fs].reshape(4, P).T.astype(np.float32)),
                "vb": np.broadcast_to(
                    wv_b[fs].reshape(1, 8, 64).astype(NPBF16), (P, 8, 64)
                ).copy(),
            }
        )
    return maps


def run_device(in_maps, time_iters=0, loop_r=None):
    """Run the SPMD kernel. Returns (per-core outp list, best wall ns or None)."""
    jitted, in_names, out_names, out_avals, zeros, mesh = _get_runner(loop_r)
    concat_in = [
        np.concatenate([in_maps[c][name] for c in range(8)], axis=0)
        for name in in_names
    ]
    args = concat_in + zeros
    outs = jitted(*args)
    outs = [np.asarray(o) for o in outs]
    best_ns = None
    if time_iters:
        import jax
        from jax.sharding import NamedSharding, PartitionSpec

        sh = NamedSharding(mesh, PartitionSpec("core"))
        dev_args = [jax.device_put(a, sh) for a in args]
        jax.block_until_ready(dev_args)
        times = []
        for _ in range(time_iters):
            t0 = time.perf_counter()
            o = jitted(*dev_args)
            jax.block_until_ready(o)
            times.append(time.perf_counter() - t0)
        best_ns = int(min(times) * 1e9)
    per_core = [
        {
            name: outs[i].reshape(8, *out_avals[i].shape)[c]
            for i, name in enumerate(out_names)
        }
        for c in range(8)
    ]
    return per_core, best_ns


def kernel(**inputs):
    x = np.asarray(inputs["x"], np.float32)
    args = {
        k: np.asarray(inputs[k], np.float32)
        for k in ["wq_w", "wq_b", "wk_w", "wk_b", "wv_w", "wv_b", "dense_w"]
    }
    in_maps = _prep_core_inputs(x, **args)
    per_core, _ = run_device(in_maps)
    dense_b = np.asarray(inputs["dense_b"], np.float32)
    out = np.empty((B, S, D), np.float32)
    for b in range(B):
        out[b] = per_core[2 * b]["outp"] + per_core[2 * b + 1]["outp"] + dense_b
    return out
